# revision 1
# baseline (speedup 1.0000x reference)
"""Trainium2 Bass kernel for nn_Eq2to2 (Maron et al. equivariant 2->2 layer).

Math (per batch n, with x[n,d,i,j] = inputs[n,i,j,d], W_b = coefs[:,:,b]):
  out[n,i,j,s] = LeakyReLU( sum_d W9[d,s] x[n,d,i,j] + W10[d,s] x[n,d,j,i]
                 + U[n,j,s] + V[n,i,s] + G[n,s] + bias[s]
                 + [i==j] (Dd[n,i,s] + E[n,s] + diag_bias[s]) )
  U = c@W5 + r@W6 + diag@W12, V = c@W7 + r@W8 + diag@W11
  Dd = diag@W0 + r@W2 + c@W3, G = tr@W13 + S@W14, E = tr@W1 + S@W4
  r = row sums, c = col sums, diag = diagonal, tr/S = their totals.

Sharding: 8 cores = (batch n = core//2) x (out-channel half = core%2).

Per core:
  build: transpose x into XT[d, i*128+j] with PE transposes (column sums
    accumulated into PSUM by a second transpose pass, row sums by chunked
    DVE reduces); input arrives host-swizzled [j, i, d] so loads are
    contiguous.
  main: per quad (4 output rows), two dense N=512 matmuls with W
    stationary in [s, j] orientation (float32r fast path); U+V merged
    off-chain on the otherwise-idle GPSIMD; one DVE add folds them into
    the PSUM result; PE transposes back to [j, s]; LeakyReLU fused into
    the PSUM->SBUF move on ACT; one contiguous 128KB DMA per quad.
  diag: the (i,i,:) outputs are recomputed exactly in a tiny [s, i] pass
    and overwrite the main loop's diagonal bytes at the end.
"""

import os
import sys

if "/opt/trn_rl_repo" not in sys.path:
    sys.path.insert(0, "/opt/trn_rl_repo")

import numpy as np

import concourse.bass as bass
import concourse.tile as tile
from concourse import bacc, mybir
from concourse.bass_utils import run_bass_kernel_spmd

B, M, D, S = 4, 128, 128, 128
SH = S // 2          # out channels per core
NB = 15              # basis size
NCORES = 8
F32 = mybir.dt.float32
AF = mybir.ActivationFunctionType
NEG_SLOPE = 0.01

# "f32" (exact) or "f32r" (fp32 storage, reduced-precision PE multiply, 4x
# matmul throughput at N>=256). Applies to the dense per-tile matmuls only.
MM_DTYPE = os.environ.get("EQ2_MM_DTYPE", "f32r")
# dummy matmuls per pair to keep the PE HAM clock-gate warm (0 = off)
WARM_MM = int(os.environ.get("EQ2_WARM", "0"))


def _xtdt():
    if MM_DTYPE == "f32r":
        return mybir.dt.float32r
    if MM_DTYPE == "bf16":
        return mybir.dt.bfloat16
    return F32


def _build_kernel():
    nc = bacc.Bacc(
        "TRN2", target_bir_lowering=False, debug=False, num_devices=NCORES
    )
    # xn is host-swizzled to [j, i, d] so build loads are contiguous
    xn = nc.dram_tensor("xn", [M, M, D], F32, kind="ExternalInput")
    wmat = nc.dram_tensor("wmat", [D, NB * SH], F32, kind="ExternalInput")
    biasv = nc.dram_tensor("biasv", [SH, 1], F32, kind="ExternalInput")
    dbiasv = nc.dram_tensor("dbiasv", [SH, 1], F32, kind="ExternalInput")
    identd = nc.dram_tensor("identd", [M, M], F32, kind="ExternalInput")
    out_t = nc.dram_tensor("out", [M, M, SH], F32, kind="ExternalOutput")

    with tile.TileContext(nc) as tc:
        _kernel_body(tc, nc, xn, wmat, biasv, dbiasv, identd, out_t)

    nc.compile()
    return nc


def _kernel_body(tc, nc, xn, wmat, biasv, dbiasv, identd, out_t):
    with (
        tc.tile_pool(name="const", bufs=1) as constp,
        tc.tile_pool(name="small", bufs=1) as smallp,
        tc.tile_pool(name="xt", bufs=1) as xtp,
    ):
        ident = constp.tile([M, M], F32)
        nc.sync.dma_start(ident[:], identd.ap())
        w_sb = constp.tile([D, NB * SH], F32)
        nc.sync.dma_start(w_sb[:], wmat.ap())
        bias_sb = smallp.tile([SH, 1], F32)
        nc.sync.dma_start(bias_sb[:], biasv.ap())
        dbias_sb = smallp.tile([SH, 1], F32)
        nc.sync.dma_start(dbias_sb[:], dbiasv.ap())

        def w(b):
            return w_sb[:, b * SH:(b + 1) * SH]

        xt = xtp.tile([D, M * M], _xtdt())   # [d, i*128 + j]
        r_sb = smallp.tile([D, M], F32)      # row sums r[n,i,d] as [d, i]
        c_sb = smallp.tile([D, M], F32)      # col sums c[n,j,d] as [d, j]
        diagT = smallp.tile([D, M], F32)     # diag[n,k,d] as [d, k]
        trace_col = smallp.tile([D, 1], F32)
        s_col = smallp.tile([D, 1], F32)
        vb_sb = smallp.tile([SH, M], F32)    # V + G + bias, [s, i]
        dcomb_sb = smallp.tile([SH, M], F32)  # Dd + E + diag_bias, [s, i]
        u_sb = smallp.tile([SH, M], F32)     # U as [s, j]
        w_r = smallp.tile([D, 2 * SH], _xtdt())  # rounded W9|W10 for mains
        wsum_sb = smallp.tile([D, SH], F32)  # W9 + W10 (diag pass)
        idr = smallp.tile([SH, SH], _xtdt())  # identity for out-transposes

        # ---- build phase: transpose x into XT, reduce r/c/diag ----
        NCH, CH = 8, M // 8
        with (
            tc.tile_pool(name="ach", bufs=4) as apool,
            tc.tile_pool(name="pt", bufs=6, space="PSUM") as ptp,
            tc.tile_pool(name="pacc", bufs=1, space="PSUM") as paccp,
        ):
            # warm the PE HAM clock-gate while the first DMAs land
            if True:
                pwu = ptp.tile([M, M], F32, tag="pt")
                for _ in range(14):
                    nc.tensor.matmul(
                        pwu[:], ident[:], ident[:], start=True, stop=True,
                        skip_group_check=True,
                    )
            psum_c = paccp.tile([D, M], F32)
            for k in range(NCH):
                ach = apool.tile([M, CH * D], F32)  # [j, (i_local, d)]
                # xn is [j, i, d]: contiguous 8KB runs per partition
                src = xn.ap()[:, k * CH:(k + 1) * CH, :]
                a3 = ach[:].rearrange("j (i d) -> j i d", i=CH)
                nc.sync.dma_start(a3, src)
                for il in range(CH):
                    i = k * CH + il
                    a_i = a3[:, il, :]  # [j=128, d=128]
                    pt = ptp.tile([D, M], F32)
                    nc.tensor.transpose(pt[:], a_i, ident[:])
                    # col-sum accumulation: psum_c += transpose(a_i)
                    nc.tensor.matmul(
                        psum_c[:], a_i, ident[:],
                        is_transpose=True,
                        start=(i == 0), stop=(i == M - 1),
                    )
                    # PSUM -> SBUF copy, alternating engines
                    dstc = xt[:, i * M:(i + 1) * M]
                    if i % 2 == 0:
                        nc.scalar.activation(dstc, pt[:], AF.Identity)
                    else:
                        nc.vector.tensor_copy(dstc, pt[:])
                # row sums r[n,i,d] as [d,i], fine-grained
                xt3 = xt[:].rearrange("d (i j) -> d i j", i=M)
                h = CH // 2
                for q in range(2):
                    lo = k * CH + q * h
                    nc.vector.reduce_sum(
                        r_sb[:, lo:lo + h],
                        xt3[:, lo:lo + h, :],
                        axis=mybir.AxisListType.X,
                    )

            nc.vector.tensor_copy(c_sb[:], psum_c[:])
            nc.vector.reduce_sum(
                s_col[:], c_sb[:], axis=mybir.AxisListType.X
            )

            # diagonal: strided DMA straight from DRAM ([j,i,d] symmetric)
            diag_nat = smallp.tile([M, D], F32)
            diag_src = bass.AP(xn, 0, [[M * D + D, M], [1, D]])
            nc.sync.dma_start(diag_nat[:], diag_src)
            pdt = ptp.tile([D, M], F32, bufs=1)
            nc.tensor.transpose(pdt[:], diag_nat[:], ident[:])
            nc.scalar.activation(
                diagT[:], pdt[:], AF.Identity, accum_out=trace_col[:]
            )

        # ---- projections of the reduced quantities ----
        with tc.tile_pool(name="proj", bufs=1, space="PSUM") as projp:
            pu = projp.tile([SH, M], F32)
            nc.tensor.matmul(pu[:], w(5), c_sb[:], start=True, stop=False)
            nc.tensor.matmul(pu[:], w(6), r_sb[:], start=False, stop=False)
            nc.tensor.matmul(pu[:], w(12), diagT[:], start=False, stop=True)

            pv = projp.tile([SH, M], F32)
            nc.tensor.matmul(pv[:], w(7), c_sb[:], start=True, stop=False)
            nc.tensor.matmul(pv[:], w(8), r_sb[:], start=False, stop=False)
            nc.tensor.matmul(pv[:], w(11), diagT[:], start=False, stop=True)

            pdd = projp.tile([SH, M], F32)
            nc.tensor.matmul(pdd[:], w(0), diagT[:], start=True, stop=False)
            nc.tensor.matmul(pdd[:], w(2), r_sb[:], start=False, stop=False)
            nc.tensor.matmul(pdd[:], w(3), c_sb[:], start=False, stop=True)

            pge = projp.tile([SH, 2], F32)
            nc.tensor.matmul(
                pge[:, 0:1], w(13), trace_col[:], start=True, stop=False)
            nc.tensor.matmul(
                pge[:, 0:1], w(14), s_col[:], start=False, stop=True)
            nc.tensor.matmul(
                pge[:, 1:2], w(1), trace_col[:], start=True, stop=False)
            nc.tensor.matmul(
                pge[:, 1:2], w(4), s_col[:], start=False, stop=True)

            gb = smallp.tile([SH, 1], F32)
            nc.vector.tensor_add(gb[:], pge[:, 0:1], bias_sb[:])
            ed = smallp.tile([SH, 1], F32)
            nc.vector.tensor_add(ed[:], pge[:, 1:2], dbias_sb[:])
            nc.vector.tensor_scalar_add(vb_sb[:], pv[:], gb[:])
            nc.vector.tensor_scalar_add(dcomb_sb[:], pdd[:], ed[:])
            nc.vector.tensor_copy(u_sb[:], pu[:])
            nc.vector.tensor_copy(w_r[:], w_sb[:, 9 * SH:11 * SH])
            nc.vector.tensor_copy(idr[:], ident[0:SH, 0:SH])
            nc.vector.tensor_add(wsum_sb[:], w(9), w(10))

        # ---- diagonal pass: exact (i,i,:) outputs in [s, i] form ----
        with tc.tile_pool(name="dg", bufs=1, space="PSUM") as dgp:
            pdg = dgp.tile([SH, M], F32)
            nc.tensor.matmul(
                pdg[:], wsum_sb[:], diagT[:], start=True, stop=True
            )
            dtmp = smallp.tile([SH, M], F32)
            nc.vector.tensor_add(dtmp[:], pdg[:], u_sb[:])
            nc.vector.tensor_add(dtmp[:], dtmp[:], vb_sb[:])
            nc.vector.tensor_add(dtmp[:], dtmp[:], dcomb_sb[:])
            pdg2 = dgp.tile([M, SH], F32)
            nc.tensor.transpose(pdg2[:], dtmp[:], ident[0:SH, 0:SH])
            dout = smallp.tile([M, SH], F32)
            nc.scalar.activation(dout[:], pdg2[:], AF.Lrelu, alpha=NEG_SLOPE)

        # ---- main loop: four output rows (one quad) per iteration ----
        xt_mm1 = xt[:].rearrange("d (i j) -> d i j", i=M)
        xt_mm2 = xt[:].rearrange("d (j i) -> d i j", j=M)
        u4 = u_sb[:].unsqueeze(1).broadcast_to([SH, 4, M])
        with (
            tc.tile_pool(name="p1", bufs=4, space="PSUM") as p1pool,
            tc.tile_pool(name="p2", bufs=3, space="PSUM") as p2pool,
            tc.tile_pool(name="tmp", bufs=4) as tmppool,
            tc.tile_pool(name="uq", bufs=3) as uqpool,
            tc.tile_pool(name="osb", bufs=4) as opool,
        ):
            for qd in range(M // 4):
                i0 = 4 * qd
                # off-chain on GPSIMD: uq = U + V columns for this quad
                uq = uqpool.tile([SH, 4 * M], F32)
                vbq = vb_sb[:, i0:i0 + 4].unsqueeze(2).broadcast_to(
                    [SH, 4, M]
                )
                nc.gpsimd.tensor_add(
                    uq[:].rearrange("s (t j) -> s t j", t=4), vbq, u4
                )
                p1 = p1pool.tile([SH, 4 * M], F32)
                p13 = p1[:].rearrange("s (t j) -> s t j", t=4)
                nc.tensor.matmul(
                    p13, w_r[:, 0:SH], xt_mm1[:, i0:i0 + 4, :],
                    start=True, stop=False,
                )
                nc.tensor.matmul(
                    p13, w_r[:, SH:2 * SH], xt_mm2[:, i0:i0 + 4, :],
                    start=False, stop=True,
                )
                # single DVE pass: tmp = psum + (U + V)
                tmp = tmppool.tile([SH, 4 * M], _xtdt())
                nc.vector.tensor_add(tmp[:], p1[:], uq[:])
                p2 = p2pool.tile([M, 4 * SH], _xtdt())
                for t in range(4):
                    nc.tensor.transpose(
                        p2[:, t * SH:(t + 1) * SH],
                        tmp[:, t * M:(t + 1) * M],
                        idr[:],
                    )
                osb = opool.tile([M, 4 * SH], F32)
                # LeakyReLU fused into the PSUM->SBUF move, on ACT
                nc.scalar.activation(
                    osb[:], p2[:], AF.Lrelu, alpha=NEG_SLOPE
                )
                dst = out_t.ap()[i0:i0 + 4, :, :].rearrange("i j s -> j i s")
                nc.sync.dma_start(
                    dst, osb[:].rearrange("j (t s) -> j t s", t=4)
                )

            # overwrite diagonal entries with the exact values
            ddst = bass.AP(out_t, 0, [[M * SH + SH, M], [1, SH]])
            nc.sync.dma_start(ddst, dout[:])


_CACHE = {}


def _get_nc():
    key = MM_DTYPE
    if key not in _CACHE:
        _CACHE[key] = _build_kernel()
    return _CACHE[key]


def make_in_maps(inputs, coefs, bias, diag_bias):
    eye = np.ascontiguousarray(np.eye(M, dtype=np.float32))
    in_maps = []
    for core in range(NCORES):
        n, sh = core // 2, core % 2
        so = sh * SH
        wprep = np.ascontiguousarray(
            coefs[:, so:so + SH, :].transpose(0, 2, 1).reshape(D, NB * SH)
        )
        in_maps.append({
            # host swizzle: [i, j, d] -> [j, i, d] for contiguous DMA
            "xn": np.ascontiguousarray(inputs[n].transpose(1, 0, 2)),
            "wmat": wprep,
            "biasv": np.ascontiguousarray(bias[so:so + SH].reshape(SH, 1)),
            "dbiasv": np.ascontiguousarray(
                diag_bias[so:so + SH].reshape(SH, 1)
            ),
            "identd": eye,
        })
    return in_maps


def kernel(inputs, coefs, bias, diag_bias):
    inputs = np.ascontiguousarray(np.asarray(inputs, dtype=np.float32))
    coefs = np.asarray(coefs, dtype=np.float32)
    bias = np.asarray(bias, dtype=np.float32).reshape(-1)
    diag_bias = np.asarray(diag_bias, dtype=np.float32).reshape(-1)

    nc = _get_nc()
    in_maps = make_in_maps(inputs, coefs, bias, diag_bias)
    # the runtime occasionally reports a transient device-unrecoverable
    # state left over from a previous process; a retry clears it
    last_exc = None
    for attempt in range(3):
        try:
            res = run_bass_kernel_spmd(
                nc, in_maps, core_ids=list(range(NCORES))
            )
            break
        except Exception as e:  # noqa: BLE001
            last_exc = e
            import time as _time
            _time.sleep(10 * (attempt + 1))
    else:
        raise last_exc

    out = np.empty((B, M, M, S), dtype=np.float32)
    for core in range(NCORES):
        n, sh = core // 2, core % 2
        out[n, :, :, sh * SH:(sh + 1) * SH] = res.results[core]["out"]
    return out

